# revision 25
# baseline (speedup 1.0000x reference)
"""Causal self-attention (GQA + RMS-norm + RoPE) Trainium2 Bass kernel.

Sharding: 8 cores = 4 batches x 2 head-groups (tensor-parallel over heads).
Core c = 2*b + t handles batch b with Q heads [8t, 8t+8) and KV heads
[2t, 2t+2). Each core computes a partial output projection (its heads'
rows of W_out); the host sums the two partials per batch.

v3 design:
  - All PE matmuls in bf16 (1 cyc/row); f32 accumulation in PSUM.
  - Fused per-window schedule; out-proj of window w-1 is interleaved
    between attention heads of window w so the PE never waits on the
    ACT exp chain; scores are software-pipelined one j-tile ahead.
  - qT/kT/v resident in SBUF; no DRAM spills.
  - RMS sum-of-squares via fused tensor_tensor_reduce (no ACT Square);
    rsqrt by Newton iteration on GpSimd; softmax denominators
    accumulate into one [8,512] PSUM bank via selector-matmuls and
    take ONE Ln + ONE Exp per window (no activation-table thrash).
  - Causal diagonal tiles compute only the unmasked column range.
"""
import sys
sys.path.insert(0, '/opt/trn_rl_repo')
import numpy as np
import ml_dtypes

from concourse import bass, bacc, mybir, tile

f32 = mybir.dt.float32
bf16 = mybir.dt.bfloat16
BF = ml_dtypes.bfloat16

B, S, D = 4, 2048, 2048
H, HKV, HD = 16, 4, 128
HLOC = H // 2          # 8 q heads per core
KVLOC = HKV // 2       # 2 kv heads per core
SCALE = float(HD) ** -0.5
ROPE_BASE = 10000.0

NTC = S // 128         # 16 token tiles
NDT = D // 128         # 16 contraction tiles
NWIN = S // 512        # 4 query windows
AF = mybir.ActivationFunctionType
ALU = mybir.AluOpType


def _rope_tables():
    inv_freq = (1.0 / (ROPE_BASE ** (np.arange(0, HD, 2, dtype=np.float32) / HD))).astype(np.float32)
    freqs = np.arange(S, dtype=np.float32)[:, None] * inv_freq[None, :]
    cos = np.cos(freqs).astype(np.float32)
    sin = np.sin(freqs).astype(np.float32)
    cos2 = np.concatenate([cos, cos], axis=1)        # [S, 128]
    sin2 = np.concatenate([sin, -sin], axis=1)       # [S, 128]
    return cos2, sin2


def _tri_masks():
    # mask[vi][p, f] = -1e30 where kv > q on the 128-wide boundary strip of
    # diagonal tile vi: kv = 128*j + p, q = 512*w + 128*vi + f, masked iff p > f.
    m = np.zeros((4, 128, 128), dtype=np.float32)
    p = np.arange(128)[:, None]
    f = np.arange(128)[None, :]
    for vi in range(4):
        m[vi][p > f] = -1e30
    return m


def build_program():
    cos_np, sin_np = _rope_tables()
    masks_np = _tri_masks()
    selS_np = np.tile(np.eye(8, dtype=BF)[None, :, :], (128, 1, 1))  # [128,h,i]=(i==h)
    selB_np = np.broadcast_to(np.eye(8, dtype=BF)[:, :, None], (8, 8, 128)).copy()

    nc = bacc.Bacc(trn_type="TRN2")

    xt_d = nc.dram_tensor("xt", [D, S], bf16, kind="ExternalInput")
    wqkv_d = nc.dram_tensor("wqkv", [D, 1536], bf16, kind="ExternalInput")
    wo_d = nc.dram_tensor("wo", [HLOC * HD, D], bf16, kind="ExternalInput")
    out_d = nc.dram_tensor("out", [S, D], f32, kind="ExternalOutput")

    cos_d = nc.inline_tensor(cos_np, "cos_t")
    sin_d = nc.inline_tensor(sin_np, "sin_t")
    ident_d = nc.inline_tensor(np.eye(128, dtype=BF), "ident")
    ident_f_d = nc.inline_tensor(np.eye(128, dtype=np.float32), "ident_f")
    masks_d = nc.inline_tensor(masks_np, "tri_masks")
    selS_d = nc.inline_tensor(selS_np, "selS")
    selB_d = nc.inline_tensor(selB_np, "selB")

    with tile.TileContext(nc) as tc:
        with tc.tile_pool(name="cst", bufs=1) as cst, \
             tc.tile_pool(name="xs", bufs=2) as xs, \
             tc.tile_pool(name="rms", bufs=2) as rms, \
             tc.tile_pool(name="nat", bufs=2) as nat, \
             tc.tile_pool(name="qp", bufs=2) as qp, \
             tc.tile_pool(name="yp", bufs=2) as yp, \
             tc.tile_pool(name="ep", bufs=2) as ep, \
             tc.tile_pool(name="dn", bufs=2) as dn, \
             tc.tile_pool(name="ob", bufs=2) as ob, \
             tc.tile_pool(name="pA", bufs=2, space="PSUM") as pA, \
             tc.tile_pool(name="pm", bufs=2, space="PSUM") as pm, \
             tc.tile_pool(name="py", bufs=1, space="PSUM") as py, \
             tc.tile_pool(name="ps8", bufs=1, space="PSUM") as ps8:

            # ---------------- constants / residents ----------------
            cos_sb = cst.tile([128, NTC, 128], f32, tag="cos")
            sin_sb = cst.tile([128, NTC, 128], f32, tag="sin")
            ident = cst.tile([128, 128], bf16, tag="ident")
            ident_f = cst.tile([128, 128], f32, tag="ident_f")
            masks = cst.tile([128, 4, 128], f32, tag="masks")
            selS = cst.tile([128, 8, 8], bf16, tag="selS")
            selB = cst.tile([8, 8, 128], bf16, tag="selB")
            wqkv_sb = cst.tile([128, NDT, 1536], bf16, tag="wqkv")
            wo_sb = cst.tile([128, HLOC, D], bf16, tag="wo")
            kt_all = cst.tile([128, KVLOC, S], bf16, tag="kt")
            v_all = cst.tile([128, NTC, KVLOC * HD], bf16, tag="v")

            # x tile for token-tile 0 first so the PE can start ASAP
            xt_tiles = {}

            def prefetch_x(tcid, eng):
                xt_sb = xs.tile([128, NDT, 128], bf16, tag="xt", name="xt_sb")
                eng.dma_start(
                    out=xt_sb[:],
                    in_=xt_d[:, tcid * 128:(tcid + 1) * 128]
                        .rearrange("(t p) s -> p t s", p=128))
                xt_tiles[tcid] = xt_sb

            prefetch_x(0, nc.sync)
            wqkv_r = wqkv_d[:].rearrange("(t p) c -> p t c", p=128)
            for dt in range(NDT):
                eng = nc.sync if dt % 2 == 0 else nc.scalar
                eng.dma_start(out=wqkv_sb[:, dt, :], in_=wqkv_r[:, dt, :])
            nc.gpsimd.dma_start(out=cos_sb[:], in_=cos_d[:].rearrange("(t p) f -> p t f", p=128))
            nc.gpsimd.dma_start(out=sin_sb[:], in_=sin_d[:].rearrange("(t p) f -> p t f", p=128))
            nc.gpsimd.dma_start(out=ident[:], in_=ident_d[:])
            nc.gpsimd.dma_start(out=ident_f[:], in_=ident_f_d[:])
            nc.gpsimd.dma_start(out=masks[:], in_=masks_d[:].rearrange("v p f -> p v f"))
            nc.gpsimd.dma_start(out=selS[:], in_=selS_d[:])
            nc.gpsimd.dma_start(out=selB[:], in_=selB_d[:])

            # ---------------- per-tcid emission helpers ----------------
            def emit_proj_mm(tcid):
                """QKV projection matmuls for one 128-token tile."""
                if tcid in xt_tiles:
                    xt_sb = xt_tiles.pop(tcid)
                else:
                    prefetch_x(tcid, nc.sync)
                    xt_sb = xt_tiles.pop(tcid)
                kvf = nat.tile([128, 256], f32, tag="kvf", name="kvf")
                qf1 = nat.tile([128, 512], f32, tag="qf1", name="qf1")
                qf2 = nat.tile([128, 512], f32, tag="qf2", name="qf2")
                # each psum is copied to SBUF right after its accumulation so
                # the pA slot frees early (2 slots cover 3 groups per tile)
                ps_kv = pA.tile([128, 512], f32, tag="acc", name="ps_kv")
                for dt in range(NDT):
                    st, sp = dt == 0, dt == NDT - 1
                    nc.tensor.matmul(ps_kv[:], xt_sb[:, dt, :], wqkv_sb[:, dt, 1024:1536], start=st, stop=sp)
                nc.vector.tensor_copy(kvf[:], ps_kv[:, 0:256])
                nc.scalar.copy(v_all[:, tcid, :], ps_kv[:, 256:512])
                ps_q1 = pA.tile([128, 512], f32, tag="acc", name="ps_q1")
                for dt in range(NDT):
                    st, sp = dt == 0, dt == NDT - 1
                    nc.tensor.matmul(ps_q1[:], xt_sb[:, dt, :], wqkv_sb[:, dt, 0:512], start=st, stop=sp)
                nc.vector.tensor_copy(qf1[:], ps_q1[:])
                ps_q2 = pA.tile([128, 512], f32, tag="acc", name="ps_q2")
                for dt in range(NDT):
                    st, sp = dt == 0, dt == NDT - 1
                    nc.tensor.matmul(ps_q2[:], xt_sb[:, dt, :], wqkv_sb[:, dt, 512:1024], start=st, stop=sp)
                nc.vector.tensor_copy(qf2[:], ps_q2[:])
                return kvf, qf1, qf2

            def emit_rope(tcid, sbufs):
                """RMS-norm (squares on ACT, Newton rsqrt on DVE) + RoPE;
                returns nat tiles."""
                kvf, qf1, qf2 = sbufs
                sq = rms.tile([128, 1280], f32, tag="sq", name="sq", bufs=1)
                sst = rms.tile([128, 10, 1, 1], f32, tag="sst", name="sst")
                yt = rms.tile([128, 10, 1, 1], f32, tag="yt", name="yt")
                tt = rms.tile([128, 10, 1, 1], f32, tag="tt", name="tt")
                s = 1.0 / float(np.sqrt(HD))
                nc.scalar.activation(sq[:, 0:512], qf1[:], AF.Square, scale=s)
                nc.scalar.activation(sq[:, 512:1024], qf2[:], AF.Square, scale=s)
                nc.scalar.activation(sq[:, 1024:1280], kvf[:], AF.Square, scale=s)
                nc.vector.tensor_reduce(
                    sst[:, 0:8, 0, :], sq[:, 0:1024].rearrange("p (h f) -> p h f", h=8),
                    axis=mybir.AxisListType.X, op=ALU.add)
                nc.vector.tensor_reduce(
                    sst[:, 8:10, 0, :], sq[:, 1024:1280].rearrange("p (h f) -> p h f", h=2),
                    axis=mybir.AxisListType.X, op=ALU.add)
                # Newton rsqrt on DVE: y0 = 1.5 - 0.5 v; 3 iterations
                v = sst[:, :, 0, 0]
                y = yt[:, :, 0, 0]
                t = tt[:, :, 0, 0]
                nc.vector.tensor_scalar(y, v, -0.5, 1.5, ALU.mult, ALU.add)
                for _ in range(3):
                    nc.vector.tensor_mul(t, y, y)
                    nc.vector.tensor_mul(t, t, v)
                    nc.vector.tensor_scalar(t, t, -0.5, 1.5, ALU.mult, ALU.add)
                    nc.vector.tensor_mul(y, y, t)

                cosr = cos_sb[:, tcid:tcid + 1, :].rearrange("p t (x f) -> p t x f", x=2)
                sinr = sin_sb[:, tcid:tcid + 1, :].rearrange("p t (x f) -> p t x f", x=2)

                def rope_group(ps_ap, nheads, rb4, out_tile):
                    shp = [128, nheads, 2, 64]
                    p4 = ps_ap.rearrange("p (h x f) -> p h x f", h=nheads, x=2)
                    p4s = p4[:, :, ::-1, :]
                    cb = cosr.to_broadcast(shp)
                    sb_ = sinr.to_broadcast(shp)
                    rb = rb4.to_broadcast(shp)
                    t1 = nat.tile([128, 4, 2, 64], f32, tag="t1", name="t1")
                    t2 = nat.tile([128, 4, 2, 64], f32, tag="t2", name="t2")
                    t1v = t1[:, 0:nheads]
                    t2v = t2[:, 0:nheads]
                    nc.vector.tensor_mul(t1v, p4, cb)
                    nc.vector.tensor_mul(t2v, p4s, sb_)
                    nc.vector.tensor_add(t1v, t1v, t2v)
                    nc.vector.tensor_mul(
                        out_tile[:].rearrange("p (h x f) -> p h x f", h=nheads, x=2),
                        t1v, rb)

                rope_group(qf1[:], 4, yt[:, 0:4], qf1)
                rope_group(qf2[:], 4, yt[:, 4:8], qf2)
                rope_group(kvf[:], 2, yt[:, 8:10], kvf)

            def emit_tr(tcid, sbufs, qt_w):
                """PE-transpose q/k for one tile into qt_w / kt_all."""
                kvf, qf1, qf2 = sbufs
                off = (tcid % 4) * 128
                tp = pm.tile([128, 2, 512], f32, tag="pm", name="tp")
                for gi, qn in ((0, qf1), (1, qf2)):
                    for hh in range(4):
                        nc.tensor.transpose(tp[:, gi, hh * 128:(hh + 1) * 128],
                                            qn[:, hh * 128:(hh + 1) * 128], ident_f[:])
                    nc.scalar.copy(
                        qt_w[:, gi * 4:(gi + 1) * 4, off:off + 128],
                        tp[:, gi, :].rearrange("p (h s) -> p h s", h=4))
                tpk = pm.tile([128, 2, 512], f32, tag="pm", name="tpk")
                for kh in range(KVLOC):
                    nc.tensor.transpose(tpk[:, 0, kh * 128:(kh + 1) * 128],
                                        kvf[:, kh * 128:(kh + 1) * 128], ident_f[:])
                nc.scalar.copy(
                    kt_all[:, :, tcid * 128:(tcid + 1) * 128],
                    tpk[:, 0, 0:256].rearrange("p (h s) -> p h s", h=2))

            def emit_proj_window(w):
                """Full projection pipeline for window w; returns qt_w tile."""
                qt_w = qp.tile([128, HLOC, 512], bf16, tag="qtw", name="qt_w")
                sb_prev = None
                for i in range(4):
                    sbufs = emit_proj_mm(4 * w + i)
                    emit_rope(4 * w + i, sbufs)
                    if sb_prev is not None:
                        emit_tr(4 * w + i - 1, sb_prev, qt_w)
                    sb_prev = sbufs
                emit_tr(4 * w + 3, sb_prev, qt_w)
                return qt_w

            def emit_attn_window(w, qt_w, fillers):
                """Attention for window w with PE-filler work interleaved
                between heads; scores pipelined one j-tile ahead."""
                njt = 4 * w + 4
                ytn_w = yp.tile([128, HLOC, 512], bf16, tag="ytn", name="ytn_w")
                ps_s8_t = ps8.tile([8, 512], f32, tag="s8", name="ps_s8")
                fill_iter = iter(fillers)
                # front-load some filler PE work to cover the tail of the
                # projection's rope/transpose chain
                for _ in range(3):
                    f = next(fill_iter, None)
                    if f is not None:
                        f()

                npair = njt // 2
                for hq in range(HLOC):
                    kvh = hq // 4
                    ps_y = py.tile([128, 512], f32, tag="y", name="ps_y")

                    def emit_score_pair(g):
                        """Scores for j-tiles (2g, 2g+1) into one 2-bank pair
                        tile; returns (tile, [(j, s0), (j, s0)])."""
                        tile_g = pm.tile([128, 2, 512], f32, tag="pm", name="ps_sc")
                        info = []
                        for parity in (0, 1):
                            j = 2 * g + parity
                            vi = j - 4 * w
                            s0 = 128 * vi if vi >= 0 else 0
                            nc.tensor.matmul(
                                tile_g[:, parity, s0:512],
                                kt_all[:, kvh, j * 128:(j + 1) * 128],
                                qt_w[:, hq, s0:512])
                            if vi >= 0:
                                nc.vector.tensor_add(tile_g[:, parity, s0:s0 + 128],
                                                     tile_g[:, parity, s0:s0 + 128],
                                                     masks[:, vi, :])
                            info.append((j, s0))
                        return tile_g, info

                    pipe = [emit_score_pair(0)]
                    if npair > 1:
                        pipe.append(emit_score_pair(1))
                    for g in range(npair):
                        if g + 2 < npair:
                            pipe.append(emit_score_pair(g + 2))
                        tile_g, info = pipe.pop(0)
                        et = ep.tile([128, 2, 512], bf16, tag="et", name="et")
                        if info[0][1] == 0 and info[1][1] == 0:
                            # both halves full width: one fused exp
                            nc.scalar.activation(et[:], tile_g[:],
                                                 AF.Exp, scale=SCALE)
                        else:
                            for parity in (0, 1):
                                s0 = info[parity][1]
                                nc.scalar.activation(et[:, parity, s0:512],
                                                     tile_g[:, parity, s0:512],
                                                     AF.Exp, scale=SCALE)
                        for parity in (0, 1):
                            j, s0 = info[parity]
                            st, sp = j == 0, j == njt - 1
                            nc.tensor.matmul(
                                ps_y[:, s0:512],
                                v_all[:, j, kvh * 128:(kvh + 1) * 128],
                                et[:, parity, s0:512], start=st, stop=sp,
                                skip_group_check=True)
                            nc.tensor.matmul(
                                ps_s8_t[:, s0:512], selS[:, hq, :], et[:, parity, s0:512],
                                start=(hq == 0 and st), stop=(hq == HLOC - 1 and sp),
                                skip_group_check=True)
                    nc.vector.tensor_copy(ytn_w[:, hq, :], ps_y[:])
                    if hq < 6:
                        f = next(fill_iter, None)
                        if f is not None:
                            f()
                return ytn_w, ps_s8_t, list(fill_iter)

            def emit_denom(ps_s8_t):
                """One Ln + one Exp for all 8 heads' softmax denominators."""
                lg8 = dn.tile([8, 512], f32, tag="lg8", name="lg8")
                rec = dn.tile([8, 512], bf16, tag="rec", name="rec")
                nc.scalar.activation(lg8[:], ps_s8_t[:], AF.Ln)
                nc.scalar.activation(rec[:], lg8[:], AF.Exp, scale=-1.0)
                return rec

            def make_fillers(w, ytn_w, rec):
                """Chunked norm + out-proj PE work for window w (to be
                interleaved between attention heads of window w+1)."""
                fillers = []

                def norm_chunk(h0):
                    def f():
                        for hq in range(h0, h0 + 4):
                            bcp = pm.tile([128, 2, 512], f32, tag="pm", name="bcp")
                            nc.tensor.matmul(bcp[:, 0, :], selB[:, hq, :], rec[:])
                            nc.vector.tensor_mul(ytn_w[:, hq, :], ytn_w[:, hq, :], bcp[:, 0, :])
                    return f

                def og_chunk(tc_i, og0):
                    def f():
                        row0 = w * 512 + tc_i * 128
                        ps_o = pm.tile([128, 2, 512], f32, tag="pm", name="ps_o")
                        for oi, og in enumerate((og0, og0 + 1)):
                            for h in range(HLOC):
                                nc.tensor.matmul(
                                    ps_o[:, oi, :],
                                    ytn_w[:, h, tc_i * 128:(tc_i + 1) * 128],
                                    wo_sb[:, h, og * 512:(og + 1) * 512],
                                    start=(h == 0), stop=(h == HLOC - 1))
                        ot = ob.tile([128, 2, 512], f32, tag="ot", name="ot")
                        nc.vector.tensor_copy(ot[:], ps_o[:])
                        nc.gpsimd.dma_start(
                            out=out_d[row0:row0 + 128, og0 * 512:(og0 + 2) * 512],
                            in_=ot[:].rearrange("p a f -> p (a f)"))
                    return f

                fillers.append(norm_chunk(0))
                fillers.append(norm_chunk(4))
                for tc_i in range(4):
                    for og0 in (0, 2):
                        fillers.append(og_chunk(tc_i, og0))
                return fillers

            # ---------------- main schedule ----------------
            qt_cur = emit_proj_window(0)
            prev = None
            for w in range(NWIN):
                fillers = make_fillers(*prev) if prev is not None else []
                ytn_w, ps_s8_t, leftover = emit_attn_window(w, qt_cur, fillers)
                rec = emit_denom(ps_s8_t)
                # leftover filler work covers the PE while the denominator
                # Ln/Exp chain runs (critical for the last window's tail)
                for f in leftover:
                    f()
                if w == 0:
                    wo_r = wo_d[:].rearrange("(h p) c -> p h c", p=128)
                    for og in range(4):
                        nc.scalar.dma_start(out=wo_sb[:, :, og * 512:(og + 1) * 512],
                                            in_=wo_r[:, :, og * 512:(og + 1) * 512])
                if w < NWIN - 1:
                    qt_cur = emit_proj_window(w + 1)
                prev = (w, ytn_w, rec)
            # tail: norm + out-proj of the last window
            for f in make_fillers(*prev):
                f()

    nc.compile()
    return nc


_PROGRAM = None


def _get_program():
    global _PROGRAM
    if _PROGRAM is None:
        _PROGRAM = build_program()
    return _PROGRAM


def make_in_maps(x, W_qkv, W_out):
    x = np.asarray(x, dtype=np.float32)
    W_qkv = np.asarray(W_qkv, dtype=np.float32)
    W_out = np.asarray(W_out, dtype=np.float32)
    in_maps = []
    for c in range(8):
        b, t = c // 2, c % 2
        xt = np.ascontiguousarray(x[b].T).astype(BF)
        wq = W_qkv[:, t * 1024:(t + 1) * 1024]
        wk = W_qkv[:, D + t * 256: D + (t + 1) * 256]
        wv = W_qkv[:, D + 512 + t * 256: D + 512 + (t + 1) * 256]
        wqkv = np.ascontiguousarray(
            np.concatenate([wq, wk, wv], axis=1)).astype(BF)
        wo = np.ascontiguousarray(W_out[t * 1024:(t + 1) * 1024, :]).astype(BF)
        in_maps.append({"xt": xt, "wqkv": wqkv, "wo": wo})
    return in_maps


def kernel(x, W_qkv, W_out):
    from concourse.bass_utils import run_bass_kernel_spmd
    nc = _get_program()
    in_maps = make_in_maps(x, W_qkv, W_out)
    res = run_bass_kernel_spmd(nc, in_maps, list(range(8)), trace=False)
    out = np.empty((B, S, D), dtype=np.float32)
    for b in range(B):
        out[b] = res.results[2 * b]["out"] + res.results[2 * b + 1]["out"]
    return out
